# revision 18
# baseline (speedup 1.0000x reference)
"""Two-layer GCN forward on 8 Trainium2 NeuronCores (Bass/Tile).

Strategy (graph/data parallel, dst-sharded), v2:
  - Nodes sharded across 8 cores (12500/core, padded to 12544 = 98*128).
  - Per layer: sharded matmul h = x @ W writes g = dinv * h straight into the
    fp32 SBUF accumulator (acc init == self-loop term), cast bf16 rows to the
    gather table, AllGather the table in 14 slices (fired early, interleaved
    into the gather stream so the in-order Pool queue never head-of-line
    blocks on them).
  - Each core owns the edges whose dst lies in its shard. Per-edge work:
    dma_gather of g[src] rows (256B HBM reads) -> SBUF messages; a one-hot
    matrix built on the vector engine (dst_local == iota) turns the
    scatter-add into PE matmuls.
  - Matmuls of one (chunk, block) piece accumulate in a PSUM tile flushed
    by one vector add into the fp32 SBUF accumulator.  (Multi-piece strips
    sharing a PSUM bank crash the HW: matmul outputs must be bank-aligned.)
  - Epilogue out = relu(dinv*acc + b) runs per 7-block slice as soon as the
    last gather phase has covered those blocks, followed immediately by the
    PE transpose + layer-2 dense matmul + layer-2 AllGather slice for that
    slice.  Layer-2 gathers for phase 0 need only the first 4 AG2 slices,
    so the layer boundary has near-zero pipeline bubble.
  - DMA queue separation: gather-index loads ride the Sync queue alone; all
    other loads/stores use the Scalar (ACT) HWDGE queue.

Edge indices are int16-limited; the table is processed in 4 row-phases of
28672 rows (= 4 AllGather slices each).  Host buckets each core's edges by
(phase, dst-block), src-sorted inside each bucket for HBM locality, with
per-(s,b) segment sizes padded to 128 and shared across cores (SPMD: one
program, per-core data).
"""

import os
import numpy as np
import ml_dtypes

import concourse.bass as bass
import concourse.tile as tile
from concourse import bacc, mybir
from concourse.bass_utils import run_bass_kernel_spmd

NC = 8           # cores
P = 128          # partitions
ROWW = 128       # bf16 feature-table row width (256 bytes)
CHUNK = 2048     # edge positions per gather call (2048 -> one call's
                 # descriptors use 1/4 of the SWDGE ring: 4 calls in flight)
NSL = 14         # AllGather slices (896 rows = 7 blocks each)
DMA_SCRATCH = 65536   # SWDGE descriptor-ring carveout (bytes)
NSWQ = 4              # SWDGE queues: queue q desc-gens on Q7 cores 2q,2q+1
SINGLE_PACKET = False

BF16 = mybir.dt.bfloat16
F32 = mybir.dt.float32
I16 = mybir.dt.int16

MSG_BUFS = 12
S_BUFS = 6
IDX_BUFS = 12
FTMP_BUFS = 8    # SBUF staging for PSUM strips (ACT copies out; DVE adds late)
FLUSH_DELAY = 6  # chunks between strip production and its DVE accumulate
STRIP_BUFS = 4   # PSUM banks for edge-accumulate strips

# crash-bisect flags
SAFE_DMA = os.environ.get("GCN_SAFE_DMA", "0") == "1"      # sync-engine DMAs only
SAFE_STRIPS = True   # multi-slot strips (sub-bank matmul offsets) crash the HW
SAFE_SCHED = os.environ.get("GCN_SAFE_SCHED", "0") == "1"  # no AG/post interleave


# ----------------------------------------------------------------- host prep

def _host_prep(x, edge_index, W1, b1, W2, b2):
    N, IN_DIM = x.shape
    HID = W1.shape[1]
    OUT = W2.shape[1]
    assert N % NC == 0
    SH = N // NC                      # real rows per shard
    SHP = -(-SH // P) * P             # padded rows per shard
    NBLK = SHP // P                   # 98
    R = NC * SHP                      # padded table rows
    SL = SHP // NSL                   # 896 rows/slice (per core)
    assert SHP % NSL == 0 and SL % P == 0
    SLICE_R = NC * SL                 # 7168 table rows per AG slice
    # variable row-phases (in AG-slice units): phase 0 needs only ONE
    # AllGather slice done before its gathers start; later phases are
    # gated while earlier ones run.  Max 4 slices = 28672 rows < int16.
    PH_SLICES = [1, 2, 4, 4, 3]
    assert sum(PH_SLICES) == NSL and max(PH_SLICES) * SLICE_R <= 32768
    bounds = np.cumsum([0] + PH_SLICES) * SLICE_R
    NPH = len(PH_SLICES)              # 5

    src = np.asarray(edge_index[0], dtype=np.int64)
    dst = np.asarray(edge_index[1], dtype=np.int64)

    deg = np.bincount(dst, minlength=N).astype(np.float64) + 1.0
    dinv = (1.0 / np.sqrt(deg)).astype(np.float32)

    # table row for node a of core c: [slice, core, rows-in-slice] layout
    sc = src // SH
    sa = src % SH
    srow = (sa // SL) * SLICE_R + sc * SL + (sa % SL)
    phase = np.searchsorted(bounds, srow, side="right") - 1
    lidx = (srow - bounds[phase]).astype(np.int16)
    core = dst // SH
    blk = (dst % SH) // P
    dlo = ((dst % SH) % P).astype(np.int16)

    # group edges per (core, phase, block); src-sorted inside for HBM row
    # locality during the gather
    order = np.lexsort((srow, blk, phase, core))
    lidx_s, phase_s, core_s, blk_s, dlo_s = (
        lidx[order], phase[order], core[order], blk[order], dlo[order])

    key = (core_s * NPH + phase_s) * NBLK + blk_s
    cnt = np.bincount(key, minlength=NC * NPH * NBLK).reshape(NC, NPH, NBLK)
    Gsb = -(-cnt.max(axis=0) // P)            # [NPH, NBLK] groups, shared

    seg_pos = {}
    posn = 0
    segments = []                             # (s, b, ngroups)
    for s in range(NPH):
        for b in range(NBLK):
            g = int(Gsb[s, b])
            if g == 0:
                continue
            seg_pos[(s, b)] = posn
            segments.append((s, b, g))
            posn += g * P
    TTOT = posn
    assert TTOT % P == 0

    # chunks: pieces of one phase, <= CHUNK positions each.
    chunks = []                               # [s, pos0, npos, [(b, ng)]]
    cur = None
    for (s, b, g) in segments:
        gleft = g
        while gleft > 0:
            if cur is not None and (cur[0] != s or cur[2] >= CHUNK):
                chunks.append(cur)
                cur = None
            if cur is None:
                cur = [s, seg_pos[(s, b)] + (g - gleft) * P, 0, []]
            take = min(gleft, (CHUNK - cur[2]) // P)
            cur[2] += take * P
            gleft -= take
            cur[3].append((b, take))
    if cur is not None:
        chunks.append(cur)

    # last chunk referencing each block (for post-work injection points)
    last_chunk = {}
    for ci, ch in enumerate(chunks):
        for (b, ng) in ch[3]:
            last_chunk[b] = ci

    # per-core position-indexed arrays
    idx_all = np.zeros((NC, TTOT), np.int16)
    dlo_all = np.full((NC, TTOT), -1.0, np.float32)
    csb_off = np.zeros(NC * NPH * NBLK + 1, np.int64)
    np.cumsum(cnt.reshape(-1), out=csb_off[1:])
    for c in range(NC):
        for s in range(NPH):
            for b in range(NBLK):
                n = int(cnt[c, s, b])
                if n == 0:
                    continue
                o = int(csb_off[(c * NPH + s) * NBLK + b])
                p0 = seg_pos[(s, b)]
                idx_all[c, p0:p0 + n] = lidx_s[o:o + n]
                dlo_all[c, p0:p0 + n] = dlo_s[o:o + n]

    # wrap gather indices: [128, TTOT/16], 16-partition wrap replicated x8
    idx_w = np.ascontiguousarray(
        np.tile(idx_all.reshape(NC, TTOT // 16, 16).transpose(0, 2, 1),
                (1, 8, 1)))
    # dst values transposed: [128, TTOT/128]
    dst_t = np.ascontiguousarray(
        dlo_all.reshape(NC, TTOT // P, P).transpose(0, 2, 1)
    ).astype(ml_dtypes.bfloat16)

    # x transposed & padded per core: [KCH, 128, SHP] bf16
    KCH = IN_DIM // P
    xtp = np.zeros((NC, KCH, P, SHP), ml_dtypes.bfloat16)
    xs = x.reshape(NC, SH, IN_DIM).astype(ml_dtypes.bfloat16)
    xtp[:, :, :, :SH] = xs.transpose(0, 2, 1).reshape(NC, KCH, P, SH)

    w1p = np.ascontiguousarray(
        W1.reshape(KCH, P, HID).transpose(1, 0, 2).reshape(P, KCH * HID)
    ).astype(ml_dtypes.bfloat16)
    w2p = np.asarray(W2).astype(ml_dtypes.bfloat16)       # [HID, OUT]

    dinv_pad = np.zeros((NC, SHP), np.float32)
    dinv_pad[:, :SH] = dinv.reshape(NC, SH)
    dinvc = np.ascontiguousarray(
        dinv_pad.reshape(NC, NBLK, P).transpose(0, 2, 1))  # [NC, 128, NBLK]

    b1r = np.tile(np.asarray(b1, np.float32)[None, :], (P, 1))
    b2r = np.tile(np.asarray(b2, np.float32)[None, :], (P, 1))
    iota = np.tile(np.arange(P, dtype=np.float32)[None, :], (P, CHUNK // P)
                   ).astype(ml_dtypes.bfloat16)
    ident = np.eye(P, dtype=np.float32)

    meta = dict(N=N, IN_DIM=IN_DIM, HID=HID, OUT=OUT, SH=SH, SHP=SHP,
                NBLK=NBLK, R=R, NPH=NPH, KCH=KCH, TTOT=TTOT, SL=SL,
                SLICE_R=SLICE_R, bounds=[int(x) for x in bounds],
                chunks=chunks, last_chunk=last_chunk)
    in_maps = []
    for c in range(NC):
        in_maps.append({
            "xt": np.ascontiguousarray(xtp[c]),
            "w1": w1p,
            "w2": w2p,
            "dinvc": np.ascontiguousarray(dinvc[c]),
            "b1r": b1r,
            "b2r": b2r,
            "iota": iota,
            "ident": ident,
            "gidx": idx_w[c],
            "dstv": dst_t[c],
        })
    return in_maps, meta


# ------------------------------------------------------------- device program

def _build_program(meta):
    HID, OUT = meta["HID"], meta["OUT"]
    SHP, NBLK, R, KCH, TTOT = (meta["SHP"], meta["NBLK"], meta["R"],
                               meta["KCH"], meta["TTOT"])
    SL, SLICE_R, NPH = meta["SL"], meta["SLICE_R"], meta["NPH"]
    bounds = meta["bounds"]
    CUM1 = bounds[1] // SLICE_R       # slices needed by phase 0 (=1)
    chunks = meta["chunks"]
    last_chunk = meta["last_chunk"]
    NCH = len(chunks)
    SPB = SL // P                    # blocks per slice (7)

    nc = bacc.Bacc("TRN2", target_bir_lowering=False, debug=False,
                   num_devices=NC, dynamic_dma_scratch_size=DMA_SCRATCH,
                   num_swdge_queues=NSWQ)

    def _dma(b):
        return b.sync.dma_start if SAFE_DMA else b.scalar.dma_start

    t_xt = nc.dram_tensor("xt", [KCH, P, SHP], BF16, kind="ExternalInput")
    t_w1 = nc.dram_tensor("w1", [P, KCH * HID], BF16, kind="ExternalInput")
    t_w2 = nc.dram_tensor("w2", [HID, OUT], BF16, kind="ExternalInput")
    t_dinvc = nc.dram_tensor("dinvc", [P, NBLK], F32, kind="ExternalInput")
    t_b1r = nc.dram_tensor("b1r", [P, HID], F32, kind="ExternalInput")
    t_b2r = nc.dram_tensor("b2r", [P, OUT], F32, kind="ExternalInput")
    t_iota = nc.dram_tensor("iota", [P, CHUNK], BF16, kind="ExternalInput")
    t_ident = nc.dram_tensor("ident", [P, P], F32, kind="ExternalInput")
    t_gidx = nc.dram_tensor("gidx", [P, TTOT // 16], I16, kind="ExternalInput")
    t_dstv = nc.dram_tensor("dstv", [P, TTOT // P], BF16, kind="ExternalInput")
    t_out = nc.dram_tensor("out", [SHP, OUT], F32, kind="ExternalOutput")

    g1_c = nc.dram_tensor("g1_c", [SHP, ROWW], BF16)
    g1_full = nc.dram_tensor("g1_full", [R, ROWW], BF16, addr_space="Shared")
    g2_c = nc.dram_tensor("g2_c", [SHP, ROWW], BF16)
    g2_full = nc.dram_tensor("g2_full", [R, ROWW], BF16, addr_space="Shared")

    with tile.TileContext(nc) as tc:
        with (tc.tile_pool(name="persist", bufs=1) as pers,
              tc.tile_pool(name="locs", bufs=1) as locs,
              tc.tile_pool(name="idx", bufs=IDX_BUFS) as idxp,
              tc.tile_pool(name="msg", bufs=MSG_BUFS) as msgp,
              tc.tile_pool(name="S", bufs=S_BUFS) as sp,
              tc.tile_pool(name="eps", bufs=3) as epp,
              tc.tile_pool(name="ftmp", bufs=FTMP_BUFS) as ftp,
              tc.tile_pool(name="stg", bufs=3) as stg,
              tc.tile_pool(name="xw", bufs=1) as xwp,
              tc.tile_pool(name="strip", bufs=STRIP_BUFS, space="PSUM") as stp,
              tc.tile_pool(name="mmps", bufs=2, space="PSUM") as mmpsp,
              tc.tile_pool(name="m2ps", bufs=1, space="PSUM") as m2psp,
              tc.tile_pool(name="tps", bufs=1, space="PSUM") as tpsp):

            w1_sb = pers.tile([P, KCH * HID], BF16)
            _dma(nc)(w1_sb[:], t_w1[:])
            w2_sb = pers.tile([HID, OUT], BF16)
            _dma(nc)(w2_sb[:], t_w2[:])
            dinv_sb = pers.tile([P, NBLK], F32)
            _dma(nc)(dinv_sb[:], t_dinvc[:])
            b1_sb = pers.tile([P, HID], F32)
            _dma(nc)(b1_sb[:], t_b1r[:])
            b2_sb = pers.tile([P, OUT], F32)
            _dma(nc)(b2_sb[:], t_b2r[:])
            iota_sb = pers.tile([P, CHUNK], BF16)
            _dma(nc)(iota_sb[:], t_iota[:])
            ident_sb = pers.tile([P, P], F32)
            _dma(nc)(ident_sb[:], t_ident[:])
            dst_sb = pers.tile([P, TTOT // P], BF16)
            _dma(nc)(dst_sb[:], t_dstv[:])

            acc1 = locs.tile([P, NBLK * HID], F32)
            acc2 = locs.tile([P, NBLK * OUT], F32)

            def ag(src_d, dst_d, si):
                nc.gpsimd.collective_compute(
                    "AllGather", mybir.AluOpType.bypass,
                    replica_groups=[list(range(NC))],
                    ins=[src_d[si * SL:(si + 1) * SL, :]],
                    outs=[dst_d[si * SLICE_R:(si + 1) * SLICE_R, :]],
                )

            # ---- layer-1 dense matmul: acc1 slice = dinv * (x @ W1);
            #      g1_c rows = the same, bf16
            for si in range(NSL):
                xw = []
                for k in range(KCH):
                    xt_k = xwp.tile([P, SL], BF16, name=f"xw{si}_{k}",
                                    tag=f"xw{k}")
                    _dma(nc)(xt_k[:],
                                        t_xt[k, :, si * SL:(si + 1) * SL])
                    xw.append(xt_k)
                for rb in range(SPB):
                    gb = si * SPB + rb
                    ps = mmpsp.tile([P, HID], F32, space="PSUM",
                                    name=f"m1ps{gb}", tag="m1ps")
                    for k in range(KCH):
                        nc.tensor.matmul(
                            ps[:],
                            lhsT=xw[k][:, rb * P:(rb + 1) * P],
                            rhs=w1_sb[:, k * HID:(k + 1) * HID],
                            start=(k == 0),
                            stop=(k == KCH - 1),
                        )
                    nc.scalar.mul(acc1[:, gb * HID:(gb + 1) * HID], ps[:],
                                  dinv_sb[:, gb:gb + 1])
                    g1b = stg.tile([P, HID], BF16, name=f"g1b{gb}", tag="g1b")
                    nc.scalar.copy(g1b[:], acc1[:, gb * HID:(gb + 1) * HID])
                    _dma(nc)(g1_c[gb * P:(gb + 1) * P, 0:HID],
                                        g1b[:])

            # ---- edge machinery -------------------------------------------
            qctr = [0]
            pending_flush = []

            def emit_chunk(layer, g_full, F, acc, ci, ch):
                s, pos0, npos, pieces = ch
                row0 = bounds[s]
                row1 = bounds[s + 1]
                idx_t = idxp.tile([P, npos // 16], I16,
                                  name=f"idx{layer}_{ci}", tag="idx")
                nc.sync.dma_start(
                    idx_t[:], t_gidx[:, pos0 // 16:(pos0 + npos) // 16])
                msgs = msgp.tile([P, npos // P, ROWW], BF16,
                                 name=f"msg{layer}_{ci}", tag="msgs")
                nc.gpsimd.dma_gather(
                    out_ap=msgs[:],
                    in_ap=g_full[row0:row1, :],
                    idxs_ap=idx_t[:],
                    num_idxs=npos,
                    num_idxs_reg=npos,
                    elem_size=ROWW,
                    single_packet=SINGLE_PACKET,
                    queue_num=qctr[0] % NSWQ,
                )
                qctr[0] += 1
                ngr = npos // P
                S = sp.tile([P, npos], BF16, name=f"S{layer}_{ci}", tag="S")
                nc.vector.tensor_tensor(
                    out=S[:].rearrange("p (g j) -> p g j", j=P),
                    in0=dst_sb[:, pos0 // P:pos0 // P + ngr
                               ].to_broadcast([P, ngr, P]),
                    in1=iota_sb[:, :npos].rearrange("p (g j) -> p g j", j=P),
                    op=mybir.AluOpType.is_equal,
                )
                # strips: runs of consecutive blocks, <= 512 fp32 per strip
                slots = 1 if SAFE_STRIPS else 512 // F
                runs = []
                for (b, ng) in pieces:
                    if (runs and len(runs[-1]) < slots
                            and runs[-1][-1][0] + 1 == b):
                        runs[-1].append((b, ng))
                    else:
                        runs.append([(b, ng)])
                while pending_flush and pending_flush[0][0] <= qctr[0] - FLUSH_DELAY:
                    pending_flush.pop(0)[1]()
                g = 0
                for ri, run in enumerate(runs):
                    strip = stp.tile([P, len(run) * F], F32, space="PSUM",
                                     name=f"st{layer}_{ci}_{ri}", tag="strip")
                    for sl, (b, ng) in enumerate(run):
                        for i in range(ng):
                            nc.tensor.matmul(
                                strip[:, sl * F:(sl + 1) * F],
                                lhsT=S[:, (g + i) * P:(g + i + 1) * P],
                                rhs=msgs[:, g + i, :F],
                                start=(i == 0),
                                stop=(i == ng - 1),
                            )
                        g += ng
                    b0 = run[0][0]
                    nr = len(run)
                    ft = ftp.tile([P, nr * F], F32, name=f"ft{layer}_{ci}_{ri}",
                                  tag="ftmp")
                    nc.vector.tensor_copy(ft[:], strip[:])
                    def _flush(acc=acc, b0=b0, nr=nr, F=F, ft=ft):
                        nc.vector.tensor_tensor(
                            acc[:, b0 * F:(b0 + nr) * F],
                            acc[:, b0 * F:(b0 + nr) * F], ft[:],
                            op=mybir.AluOpType.add)
                    pending_flush.append((qctr[0], _flush))

            def l1_post_slice(si):
                # epilogue out1 = relu(dinv*acc1 + b1) for slice si, then
                # transpose + layer-2 dense matmul + acc2 init, then AG2(si)
                while pending_flush:
                    pending_flush.pop(0)[1]()
                lo, hi = si * SPB * HID, (si + 1) * SPB * HID
                a3 = acc1[:, lo:hi].rearrange("p (n h) -> p n h", h=HID)
                nc.vector.tensor_tensor(
                    out=a3, in0=a3,
                    in1=dinv_sb[:, si * SPB:(si + 1) * SPB
                                ].to_broadcast([P, SPB, HID]),
                    op=mybir.AluOpType.mult)
                nc.vector.tensor_tensor(
                    out=a3, in0=a3,
                    in1=b1_sb[:].to_broadcast([P, HID, SPB]
                                              ).rearrange("p h n -> p n h"),
                    op=mybir.AluOpType.add)
                nc.scalar.activation(acc1[:, lo:hi], acc1[:, lo:hi],
                                     mybir.ActivationFunctionType.Relu)
                for rb in range(SPB):
                    b = si * SPB + rb
                    pst = tpsp.tile([HID, P], F32, space="PSUM",
                                    name=f"pst{b}", tag="pst")
                    nc.tensor.transpose(
                        pst[:], acc1[:, b * HID:(b + 1) * HID], ident_sb[:])
                    o1s = stg.tile([HID, P], BF16, name=f"o1s{b}", tag="o1s")
                    nc.scalar.copy(o1s[:], pst[:])
                    ps2 = m2psp.tile([P, OUT], F32, space="PSUM",
                                     name=f"m2ps{b}", tag="m2ps")
                    nc.tensor.matmul(ps2[:], lhsT=o1s[:],
                                     rhs=w2_sb[:], start=True, stop=True)
                    nc.scalar.mul(acc2[:, b * OUT:(b + 1) * OUT], ps2[:],
                                  dinv_sb[:, b:b + 1])
                    g2b = stg.tile([P, OUT], BF16, name=f"g2b{b}", tag="g2b")
                    nc.scalar.mul(g2b[:], ps2[:], dinv_sb[:, b:b + 1])
                    _dma(nc)(g2_c[b * P:(b + 1) * P, 0:OUT],
                                        g2b[:])

            def l2_post_slice(si):
                while pending_flush:
                    pending_flush.pop(0)[1]()
                lo, hi = si * SPB * OUT, (si + 1) * SPB * OUT
                c3 = acc2[:, lo:hi].rearrange("p (n o) -> p n o", o=OUT)
                nc.vector.tensor_tensor(
                    out=c3, in0=c3,
                    in1=dinv_sb[:, si * SPB:(si + 1) * SPB
                                ].to_broadcast([P, SPB, OUT]),
                    op=mybir.AluOpType.mult)
                nc.vector.tensor_tensor(
                    out=c3, in0=c3,
                    in1=b2_sb[:].to_broadcast([P, OUT, SPB]
                                              ).rearrange("p h n -> p n h"),
                    op=mybir.AluOpType.add)
                for rb in range(SPB):
                    b = si * SPB + rb
                    _dma(nc)(
                        t_out[b * P:(b + 1) * P, :],
                        acc2[:, b * OUT:(b + 1) * OUT])

            # ---- event schedule over the global chunk stream --------------
            # positions: 0..NCH-1 = layer-1 chunks, NCH..2*NCH-1 = layer-2
            # chunks; events fire after the chunk at their position.
            slice_avail1 = {}
            for si in range(NSL):
                slice_avail1[si] = max(last_chunk[si * SPB + rb]
                                       for rb in range(SPB))
            events = {}

            def add_event(pos, fn):
                events.setdefault(pos, []).append(fn)

            # AG1 slices CUM1..13 interleave into the layer-1 stream
            for j, si in enumerate(range(CUM1, NSL)):
                add_event(2 * j + 1, lambda si=si: ag(g1_c, g1_full, si))
            # layer-1 post-slice work as soon as the last phase covers the
            # slice; AG2 for slices 0-3 must fire before the first layer-2
            # chunk, the rest interleave into the layer-2 stream
            pos = 0
            for si in range(NSL):
                pos = max(slice_avail1[si] + 3, pos + 1)
                add_event(min(pos, NCH - 1), lambda si=si: l1_post_slice(si))
                if si < CUM1:
                    agpos = min(pos + 2, NCH - 1)
                else:
                    agpos = max(pos + 2, NCH + 2 * (si - CUM1))
                add_event(agpos, lambda si=si: ag(g2_c, g2_full, si))
            # layer-2 post-slice work
            pos = NCH
            for si in range(NSL):
                pos = max(NCH + slice_avail1[si] + 3, pos + 1)
                add_event(min(pos, 2 * NCH - 1),
                          lambda si=si: l2_post_slice(si))

            # ---- run the stream -------------------------------------------
            for si in range(4):
                ag(g1_c, g1_full, si)
            for ci, ch in enumerate(chunks):
                emit_chunk(1, g1_full, HID, acc1, ci, ch)
                for fn in events.get(ci, []):
                    fn()
            for ci, ch in enumerate(chunks):
                emit_chunk(2, g2_full, OUT, acc2, ci, ch)
                for fn in events.get(NCH + ci, []):
                    fn()
            for pos in sorted(k for k in events if k >= 2 * NCH):
                for fn in events[pos]:
                    fn()

    nc.compile()
    return nc


# ------------------------------------------------------------------ frontend

_CACHE = {}


def run(trace=False, **inputs):
    in_maps, meta = _host_prep(
        inputs["x"], inputs["edge_index"], inputs["W1"], inputs["b1"],
        inputs["W2"], inputs["b2"])
    key = (meta["N"], meta["IN_DIM"], meta["HID"], meta["OUT"], meta["TTOT"],
           tuple((s, p, n, tuple(sg)) for s, p, n, sg in meta["chunks"]))
    if key not in _CACHE:
        _CACHE.clear()
        _CACHE[key] = _build_program(meta)
    nc = _CACHE[key]
    res = run_bass_kernel_spmd(nc, in_maps, list(range(NC)), trace=trace)
    SH = meta["SH"]
    out = np.concatenate([res.results[c]["out"][:SH] for c in range(NC)],
                         axis=0)
    return out.astype(np.float32), res


def kernel(**inputs):
    out, _ = run(trace=False, **inputs)
    return out


# revision 20
# speedup vs baseline: 1.1991x; 1.1991x over previous
"""Two-layer GCN forward on 8 Trainium2 NeuronCores (Bass/Tile).

Strategy (graph/data parallel, dst-sharded), v2:
  - Nodes sharded across 8 cores (12500/core, padded to 12544 = 98*128).
  - Per layer: sharded matmul h = x @ W writes g = dinv * h straight into the
    fp32 SBUF accumulator (acc init == self-loop term), cast bf16 rows to the
    gather table, AllGather the table in 14 slices (fired early, interleaved
    into the gather stream so the in-order Pool queue never head-of-line
    blocks on them).
  - Each core owns the edges whose dst lies in its shard. Per-edge work:
    dma_gather of g[src] rows (256B HBM reads) -> SBUF messages; a one-hot
    matrix built on the vector engine (dst_local == iota) turns the
    scatter-add into PE matmuls.
  - Matmuls of one (chunk, block) piece accumulate in a PSUM tile flushed
    by one vector add into the fp32 SBUF accumulator.  (Multi-piece strips
    sharing a PSUM bank crash the HW: matmul outputs must be bank-aligned.)
  - Epilogue out = relu(dinv*acc + b) runs per 7-block slice as soon as the
    last gather phase has covered those blocks, followed immediately by the
    PE transpose + layer-2 dense matmul + layer-2 AllGather slice for that
    slice.  Layer-2 gathers for phase 0 need only the first 4 AG2 slices,
    so the layer boundary has near-zero pipeline bubble.
  - DMA queue separation: gather-index loads ride the Sync queue alone; all
    other loads/stores use the Scalar (ACT) HWDGE queue.

Edge indices are int16-limited; the table is processed in 4 row-phases of
28672 rows (= 4 AllGather slices each).  Host buckets each core's edges by
(phase, dst-block), src-sorted inside each bucket for HBM locality, with
per-(s,b) segment sizes padded to 128 and shared across cores (SPMD: one
program, per-core data).
"""

import os
import numpy as np
import ml_dtypes

import concourse.bass as bass
import concourse.tile as tile
from concourse import bacc, mybir
from concourse.bass_utils import run_bass_kernel_spmd

NC = 8           # cores
P = 128          # partitions
ROWW = 128       # bf16 feature-table row width (256 bytes)
CHUNK = 2048     # edge positions per gather call (2048 -> one call's
                 # descriptors use 1/4 of the SWDGE ring: 4 calls in flight)
NSL = 14         # AllGather slices (896 rows = 7 blocks each)
DMA_SCRATCH = 65536   # SWDGE descriptor-ring carveout (bytes)
NSWQ = 4              # SWDGE queues: queue q desc-gens on Q7 cores 2q,2q+1
SINGLE_PACKET = False

BF16 = mybir.dt.bfloat16
F32 = mybir.dt.float32
I16 = mybir.dt.int16

MSG_BUFS = 12
S_BUFS = 6
IDX_BUFS = 12
FTMP_BUFS = 8    # SBUF staging for PSUM strips (ACT copies out; DVE adds late)
FLUSH_DELAY = 6  # chunks between strip production and its DVE accumulate
STRIP_BUFS = 4   # PSUM banks for edge-accumulate strips

# crash-bisect flags
SAFE_DMA = os.environ.get("GCN_SAFE_DMA", "0") == "1"      # sync-engine DMAs only
SAFE_STRIPS = True   # multi-slot strips (sub-bank matmul offsets) crash the HW
SAFE_SCHED = os.environ.get("GCN_SAFE_SCHED", "0") == "1"  # no AG/post interleave


# ----------------------------------------------------------------- host prep

def _host_prep(x, edge_index, W1, b1, W2, b2):
    N, IN_DIM = x.shape
    HID = W1.shape[1]
    OUT = W2.shape[1]
    assert N % NC == 0
    SH = N // NC                      # real rows per shard
    SHP = -(-SH // P) * P             # padded rows per shard
    NBLK = SHP // P                   # 98
    R = NC * SHP                      # padded table rows
    SL = SHP // NSL                   # 896 rows/slice (per core)
    assert SHP % NSL == 0 and SL % P == 0
    SLICE_R = NC * SL                 # 7168 table rows per AG slice
    # variable row-phases (in AG-slice units): phase 0 needs only ONE
    # AllGather slice done before its gathers start; later phases are
    # gated while earlier ones run.  Max 4 slices = 28672 rows < int16.
    PH_SLICES = [1, 2, 4, 4, 3]
    assert sum(PH_SLICES) == NSL and max(PH_SLICES) * SLICE_R <= 32768
    bounds = np.cumsum([0] + PH_SLICES) * SLICE_R
    NPH = len(PH_SLICES)              # 5

    src = np.asarray(edge_index[0], dtype=np.int64)
    dst = np.asarray(edge_index[1], dtype=np.int64)

    deg = np.bincount(dst, minlength=N).astype(np.float64) + 1.0
    dinv = (1.0 / np.sqrt(deg)).astype(np.float32)

    # table row for node a of core c: [slice, core, rows-in-slice] layout
    sc = src // SH
    sa = src % SH
    srow = (sa // SL) * SLICE_R + sc * SL + (sa % SL)
    phase = np.searchsorted(bounds, srow, side="right") - 1
    lidx = (srow - bounds[phase]).astype(np.int16)
    core = dst // SH
    blk = (dst % SH) // P
    dlo = ((dst % SH) % P).astype(np.int16)

    # group edges per (core, phase, block); src-sorted inside for HBM row
    # locality during the gather
    order = np.lexsort((srow, blk, phase, core))
    lidx_s, phase_s, core_s, blk_s, dlo_s = (
        lidx[order], phase[order], core[order], blk[order], dlo[order])

    key = (core_s * NPH + phase_s) * NBLK + blk_s
    cnt = np.bincount(key, minlength=NC * NPH * NBLK).reshape(NC, NPH, NBLK)
    Gsb = -(-cnt.max(axis=0) // P)            # [NPH, NBLK] groups, shared

    seg_pos = {}
    posn = 0
    segments = []                             # (s, b, ngroups)
    for s in range(NPH):
        for b in range(NBLK):
            g = int(Gsb[s, b])
            if g == 0:
                continue
            seg_pos[(s, b)] = posn
            segments.append((s, b, g))
            posn += g * P
    TTOT = posn
    assert TTOT % P == 0

    # chunks: pieces of one phase, <= CHUNK positions each.
    chunks = []                               # [s, pos0, npos, [(b, ng)]]
    cur = None
    for (s, b, g) in segments:
        gleft = g
        while gleft > 0:
            if cur is not None and (cur[0] != s or cur[2] >= CHUNK):
                chunks.append(cur)
                cur = None
            if cur is None:
                cur = [s, seg_pos[(s, b)] + (g - gleft) * P, 0, []]
            take = min(gleft, (CHUNK - cur[2]) // P)
            cur[2] += take * P
            gleft -= take
            cur[3].append((b, take))
    if cur is not None:
        chunks.append(cur)

    # last chunk referencing each block (for post-work injection points)
    last_chunk = {}
    for ci, ch in enumerate(chunks):
        for (b, ng) in ch[3]:
            last_chunk[b] = ci

    # per-core position-indexed arrays
    idx_all = np.zeros((NC, TTOT), np.int16)
    dlo_all = np.full((NC, TTOT), -1.0, np.float32)
    csb_off = np.zeros(NC * NPH * NBLK + 1, np.int64)
    np.cumsum(cnt.reshape(-1), out=csb_off[1:])
    for c in range(NC):
        for s in range(NPH):
            for b in range(NBLK):
                n = int(cnt[c, s, b])
                if n == 0:
                    continue
                o = int(csb_off[(c * NPH + s) * NBLK + b])
                p0 = seg_pos[(s, b)]
                idx_all[c, p0:p0 + n] = lidx_s[o:o + n]
                dlo_all[c, p0:p0 + n] = dlo_s[o:o + n]

    # wrap gather indices: [128, TTOT/16], 16-partition wrap replicated x8
    idx_w = np.ascontiguousarray(
        np.tile(idx_all.reshape(NC, TTOT // 16, 16).transpose(0, 2, 1),
                (1, 8, 1)))
    # dst values transposed: [128, TTOT/128]
    dst_t = np.ascontiguousarray(
        dlo_all.reshape(NC, TTOT // P, P).transpose(0, 2, 1)
    ).astype(ml_dtypes.bfloat16)

    # x transposed & padded per core: [KCH, 128, SHP] bf16
    KCH = IN_DIM // P
    xtp = np.zeros((NC, KCH, P, SHP), ml_dtypes.bfloat16)
    xs = x.reshape(NC, SH, IN_DIM).astype(ml_dtypes.bfloat16)
    xtp[:, :, :, :SH] = xs.transpose(0, 2, 1).reshape(NC, KCH, P, SH)

    w1p = np.ascontiguousarray(
        W1.reshape(KCH, P, HID).transpose(1, 0, 2).reshape(P, KCH * HID)
    ).astype(ml_dtypes.bfloat16)
    w2p = np.asarray(W2).astype(ml_dtypes.bfloat16)       # [HID, OUT]

    dinv_pad = np.zeros((NC, SHP), np.float32)
    dinv_pad[:, :SH] = dinv.reshape(NC, SH)
    dinvc = np.ascontiguousarray(
        dinv_pad.reshape(NC, NBLK, P).transpose(0, 2, 1))  # [NC, 128, NBLK]

    b1r = np.tile(np.asarray(b1, np.float32)[None, :], (P, 1))
    b2r = np.tile(np.asarray(b2, np.float32)[None, :], (P, 1))
    iota = np.tile(np.arange(P, dtype=np.float32)[None, :], (P, CHUNK // P)
                   ).astype(ml_dtypes.bfloat16)
    ident = np.eye(P, dtype=np.float32)

    meta = dict(N=N, IN_DIM=IN_DIM, HID=HID, OUT=OUT, SH=SH, SHP=SHP,
                NBLK=NBLK, R=R, NPH=NPH, KCH=KCH, TTOT=TTOT, SL=SL,
                SLICE_R=SLICE_R, bounds=[int(x) for x in bounds],
                chunks=chunks, last_chunk=last_chunk)
    in_maps = []
    for c in range(NC):
        in_maps.append({
            "xt": np.ascontiguousarray(xtp[c]),
            "w1": w1p,
            "w2": w2p,
            "dinvc": np.ascontiguousarray(dinvc[c]),
            "b1r": b1r,
            "b2r": b2r,
            "iota": iota,
            "ident": ident,
            "gidx": idx_w[c],
            "dstv": dst_t[c],
        })
    return in_maps, meta


# ------------------------------------------------------------- device program

def _build_program(meta):
    HID, OUT = meta["HID"], meta["OUT"]
    SHP, NBLK, R, KCH, TTOT = (meta["SHP"], meta["NBLK"], meta["R"],
                               meta["KCH"], meta["TTOT"])
    SL, SLICE_R, NPH = meta["SL"], meta["SLICE_R"], meta["NPH"]
    bounds = meta["bounds"]
    CUM1 = bounds[1] // SLICE_R       # slices needed by phase 0 (=1)
    chunks = meta["chunks"]
    last_chunk = meta["last_chunk"]
    NCH = len(chunks)
    SPB = SL // P                    # blocks per slice (7)

    nc = bacc.Bacc("TRN2", target_bir_lowering=False, debug=False,
                   num_devices=NC, dynamic_dma_scratch_size=DMA_SCRATCH,
                   num_swdge_queues=NSWQ)

    def _dma(b):
        return b.sync.dma_start if SAFE_DMA else b.scalar.dma_start

    t_xt = nc.dram_tensor("xt", [KCH, P, SHP], BF16, kind="ExternalInput")
    t_w1 = nc.dram_tensor("w1", [P, KCH * HID], BF16, kind="ExternalInput")
    t_w2 = nc.dram_tensor("w2", [HID, OUT], BF16, kind="ExternalInput")
    t_dinvc = nc.dram_tensor("dinvc", [P, NBLK], F32, kind="ExternalInput")
    t_b1r = nc.dram_tensor("b1r", [P, HID], F32, kind="ExternalInput")
    t_b2r = nc.dram_tensor("b2r", [P, OUT], F32, kind="ExternalInput")
    t_iota = nc.dram_tensor("iota", [P, CHUNK], BF16, kind="ExternalInput")
    t_ident = nc.dram_tensor("ident", [P, P], F32, kind="ExternalInput")
    t_gidx = nc.dram_tensor("gidx", [P, TTOT // 16], I16, kind="ExternalInput")
    t_dstv = nc.dram_tensor("dstv", [P, TTOT // P], BF16, kind="ExternalInput")
    t_out = nc.dram_tensor("out", [SHP, OUT], F32, kind="ExternalOutput")

    g1_c = nc.dram_tensor("g1_c", [SHP, HID], BF16)
    g1h_full = nc.dram_tensor("g1h_full", [R, HID], BF16, addr_space="Shared")
    g1_full = nc.dram_tensor("g1_full", [R, ROWW], BF16)
    g2_c = nc.dram_tensor("g2_c", [SHP, OUT], BF16)
    g2h_full = nc.dram_tensor("g2h_full", [R, OUT], BF16, addr_space="Shared")
    g2_full = nc.dram_tensor("g2_full", [R, ROWW], BF16)

    with tile.TileContext(nc) as tc:
        with (tc.tile_pool(name="persist", bufs=1) as pers,
              tc.tile_pool(name="locs", bufs=1) as locs,
              tc.tile_pool(name="idx", bufs=IDX_BUFS) as idxp,
              tc.tile_pool(name="msg", bufs=MSG_BUFS) as msgp,
              tc.tile_pool(name="S", bufs=S_BUFS) as sp,
              tc.tile_pool(name="eps", bufs=3) as epp,
              tc.tile_pool(name="ftmp", bufs=FTMP_BUFS) as ftp,
              tc.tile_pool(name="restride", bufs=3) as rs_pool,
              tc.tile_pool(name="stg", bufs=3) as stg,
              tc.tile_pool(name="xw", bufs=1) as xwp,
              tc.tile_pool(name="strip", bufs=STRIP_BUFS, space="PSUM") as stp,
              tc.tile_pool(name="mmps", bufs=2, space="PSUM") as mmpsp,
              tc.tile_pool(name="m2ps", bufs=1, space="PSUM") as m2psp,
              tc.tile_pool(name="tps", bufs=1, space="PSUM") as tpsp):

            w1_sb = pers.tile([P, KCH * HID], BF16)
            _dma(nc)(w1_sb[:], t_w1[:])
            w2_sb = pers.tile([HID, OUT], BF16)
            _dma(nc)(w2_sb[:], t_w2[:])
            dinv_sb = pers.tile([P, NBLK], F32)
            _dma(nc)(dinv_sb[:], t_dinvc[:])
            b1_sb = pers.tile([P, HID], F32)
            _dma(nc)(b1_sb[:], t_b1r[:])
            b2_sb = pers.tile([P, OUT], F32)
            _dma(nc)(b2_sb[:], t_b2r[:])
            iota_sb = pers.tile([P, CHUNK], BF16)
            _dma(nc)(iota_sb[:], t_iota[:])
            ident_sb = pers.tile([P, P], F32)
            _dma(nc)(ident_sb[:], t_ident[:])
            dst_sb = pers.tile([P, TTOT // P], BF16)
            _dma(nc)(dst_sb[:], t_dstv[:])

            acc1 = locs.tile([P, NBLK * HID], F32)
            acc2 = locs.tile([P, NBLK * OUT], F32)

            def ag(src_d, mid_d, dst_d, F, si):
                # AllGather compact F-wide rows, then restride to the 256B
                # gather-table rows through an SBUF bounce (sync queue).
                nc.gpsimd.collective_compute(
                    "AllGather", mybir.AluOpType.bypass,
                    replica_groups=[list(range(NC))],
                    ins=[src_d[si * SL:(si + 1) * SL, :]],
                    outs=[mid_d[si * SLICE_R:(si + 1) * SLICE_R, :]],
                )
                rows = SLICE_R
                per_p = rows // P          # 56 rows per partition
                rt = rs_pool.tile([P, per_p * F], BF16, name=f"rt{dst_d.name}_{si}",
                                  tag="rt")
                nc.sync.dma_start(
                    rt[:].rearrange("p (j f) -> p j f", f=F),
                    mid_d[si * rows:(si + 1) * rows, :
                          ].rearrange("(p j) f -> p j f", p=P))
                nc.sync.dma_start(
                    g_restride_out(dst_d, si, rows, F),
                    rt[:].rearrange("p (j f) -> p j f", f=F))

            def g_restride_out(dst_d, si, rows, F):
                return dst_d[si * rows:(si + 1) * rows, 0:F
                             ].rearrange("(p j) f -> p j f", p=P)

            # ---- layer-1 dense matmul: acc1 slice = dinv * (x @ W1);
            #      g1_c rows = the same, bf16
            for si in range(NSL):
                xw = []
                for k in range(KCH):
                    xt_k = xwp.tile([P, SL], BF16, name=f"xw{si}_{k}",
                                    tag=f"xw{k}")
                    _dma(nc)(xt_k[:],
                                        t_xt[k, :, si * SL:(si + 1) * SL])
                    xw.append(xt_k)
                for rb in range(SPB):
                    gb = si * SPB + rb
                    ps = mmpsp.tile([P, HID], F32, space="PSUM",
                                    name=f"m1ps{gb}", tag="m1ps")
                    for k in range(KCH):
                        nc.tensor.matmul(
                            ps[:],
                            lhsT=xw[k][:, rb * P:(rb + 1) * P],
                            rhs=w1_sb[:, k * HID:(k + 1) * HID],
                            start=(k == 0),
                            stop=(k == KCH - 1),
                        )
                    nc.scalar.mul(acc1[:, gb * HID:(gb + 1) * HID], ps[:],
                                  dinv_sb[:, gb:gb + 1])
                    g1b = stg.tile([P, HID], BF16, name=f"g1b{gb}", tag="g1b")
                    nc.scalar.copy(g1b[:], acc1[:, gb * HID:(gb + 1) * HID])
                    _dma(nc)(g1_c[gb * P:(gb + 1) * P, 0:HID],
                                        g1b[:])

            # ---- edge machinery -------------------------------------------
            qctr = [0]
            pending_flush = []

            def emit_chunk(layer, g_full, F, acc, ci, ch):
                s, pos0, npos, pieces = ch
                row0 = bounds[s]
                row1 = bounds[s + 1]
                idx_t = idxp.tile([P, npos // 16], I16,
                                  name=f"idx{layer}_{ci}", tag="idx")
                nc.sync.dma_start(
                    idx_t[:], t_gidx[:, pos0 // 16:(pos0 + npos) // 16])
                msgs = msgp.tile([P, npos // P, ROWW], BF16,
                                 name=f"msg{layer}_{ci}", tag="msgs")
                nc.gpsimd.dma_gather(
                    out_ap=msgs[:],
                    in_ap=g_full[row0:row1, :],
                    idxs_ap=idx_t[:],
                    num_idxs=npos,
                    num_idxs_reg=npos,
                    elem_size=ROWW,
                    single_packet=SINGLE_PACKET,
                    queue_num=qctr[0] % NSWQ,
                )
                qctr[0] += 1
                ngr = npos // P
                S = sp.tile([P, npos], BF16, name=f"S{layer}_{ci}", tag="S")
                nc.vector.tensor_tensor(
                    out=S[:].rearrange("p (g j) -> p g j", j=P),
                    in0=dst_sb[:, pos0 // P:pos0 // P + ngr
                               ].to_broadcast([P, ngr, P]),
                    in1=iota_sb[:, :npos].rearrange("p (g j) -> p g j", j=P),
                    op=mybir.AluOpType.is_equal,
                )
                # strips: runs of consecutive blocks, <= 512 fp32 per strip
                slots = 1 if SAFE_STRIPS else 512 // F
                runs = []
                for (b, ng) in pieces:
                    if (runs and len(runs[-1]) < slots
                            and runs[-1][-1][0] + 1 == b):
                        runs[-1].append((b, ng))
                    else:
                        runs.append([(b, ng)])
                while pending_flush and pending_flush[0][0] <= qctr[0] - FLUSH_DELAY:
                    pending_flush.pop(0)[1]()
                g = 0
                for ri, run in enumerate(runs):
                    strip = stp.tile([P, len(run) * F], F32, space="PSUM",
                                     name=f"st{layer}_{ci}_{ri}", tag="strip")
                    for sl, (b, ng) in enumerate(run):
                        for i in range(ng):
                            nc.tensor.matmul(
                                strip[:, sl * F:(sl + 1) * F],
                                lhsT=S[:, (g + i) * P:(g + i + 1) * P],
                                rhs=msgs[:, g + i, :F],
                                start=(i == 0),
                                stop=(i == ng - 1),
                            )
                        g += ng
                    b0 = run[0][0]
                    nr = len(run)
                    ft = ftp.tile([P, nr * F], F32, name=f"ft{layer}_{ci}_{ri}",
                                  tag="ftmp")
                    nc.vector.tensor_copy(ft[:], strip[:])
                    def _flush(acc=acc, b0=b0, nr=nr, F=F, ft=ft):
                        nc.vector.tensor_tensor(
                            acc[:, b0 * F:(b0 + nr) * F],
                            acc[:, b0 * F:(b0 + nr) * F], ft[:],
                            op=mybir.AluOpType.add)
                    pending_flush.append((qctr[0], _flush))

            def l1_post_slice(si):
                # epilogue out1 = relu(dinv*acc1 + b1) for slice si, then
                # transpose + layer-2 dense matmul + acc2 init, then AG2(si)
                while pending_flush:
                    pending_flush.pop(0)[1]()
                lo, hi = si * SPB * HID, (si + 1) * SPB * HID
                a3 = acc1[:, lo:hi].rearrange("p (n h) -> p n h", h=HID)
                nc.vector.tensor_tensor(
                    out=a3, in0=a3,
                    in1=dinv_sb[:, si * SPB:(si + 1) * SPB
                                ].to_broadcast([P, SPB, HID]),
                    op=mybir.AluOpType.mult)
                nc.vector.tensor_tensor(
                    out=a3, in0=a3,
                    in1=b1_sb[:].to_broadcast([P, HID, SPB]
                                              ).rearrange("p h n -> p n h"),
                    op=mybir.AluOpType.add)
                nc.scalar.activation(acc1[:, lo:hi], acc1[:, lo:hi],
                                     mybir.ActivationFunctionType.Relu)
                for rb in range(SPB):
                    b = si * SPB + rb
                    pst = tpsp.tile([HID, P], F32, space="PSUM",
                                    name=f"pst{b}", tag="pst")
                    nc.tensor.transpose(
                        pst[:], acc1[:, b * HID:(b + 1) * HID], ident_sb[:])
                    o1s = stg.tile([HID, P], BF16, name=f"o1s{b}", tag="o1s")
                    nc.scalar.copy(o1s[:], pst[:])
                    ps2 = m2psp.tile([P, OUT], F32, space="PSUM",
                                     name=f"m2ps{b}", tag="m2ps")
                    nc.tensor.matmul(ps2[:], lhsT=o1s[:],
                                     rhs=w2_sb[:], start=True, stop=True)
                    nc.scalar.mul(acc2[:, b * OUT:(b + 1) * OUT], ps2[:],
                                  dinv_sb[:, b:b + 1])
                    g2b = stg.tile([P, OUT], BF16, name=f"g2b{b}", tag="g2b")
                    nc.scalar.mul(g2b[:], ps2[:], dinv_sb[:, b:b + 1])
                    _dma(nc)(g2_c[b * P:(b + 1) * P, 0:OUT],
                                        g2b[:])

            def l2_post_slice(si):
                while pending_flush:
                    pending_flush.pop(0)[1]()
                lo, hi = si * SPB * OUT, (si + 1) * SPB * OUT
                c3 = acc2[:, lo:hi].rearrange("p (n o) -> p n o", o=OUT)
                nc.vector.tensor_tensor(
                    out=c3, in0=c3,
                    in1=dinv_sb[:, si * SPB:(si + 1) * SPB
                                ].to_broadcast([P, SPB, OUT]),
                    op=mybir.AluOpType.mult)
                nc.vector.tensor_tensor(
                    out=c3, in0=c3,
                    in1=b2_sb[:].to_broadcast([P, OUT, SPB]
                                              ).rearrange("p h n -> p n h"),
                    op=mybir.AluOpType.add)
                for rb in range(SPB):
                    b = si * SPB + rb
                    _dma(nc)(
                        t_out[b * P:(b + 1) * P, :],
                        acc2[:, b * OUT:(b + 1) * OUT])

            # ---- event schedule over the global chunk stream --------------
            # positions: 0..NCH-1 = layer-1 chunks, NCH..2*NCH-1 = layer-2
            # chunks; events fire after the chunk at their position.
            slice_avail1 = {}
            for si in range(NSL):
                slice_avail1[si] = max(last_chunk[si * SPB + rb]
                                       for rb in range(SPB))
            events = {}

            def add_event(pos, fn):
                events.setdefault(pos, []).append(fn)

            # AG1 slices CUM1..13 interleave into the layer-1 stream
            for j, si in enumerate(range(CUM1, NSL)):
                add_event(2 * j + 1, lambda si=si: ag(g1_c, g1h_full, g1_full, HID, si))
            # layer-1 post-slice work as soon as the last phase covers the
            # slice; AG2 for slices 0-3 must fire before the first layer-2
            # chunk, the rest interleave into the layer-2 stream
            pos = 0
            for si in range(NSL):
                pos = max(slice_avail1[si] + 3, pos + 1)
                add_event(min(pos, NCH - 1), lambda si=si: l1_post_slice(si))
                if si < CUM1:
                    agpos = min(pos + 2, NCH - 1)
                else:
                    agpos = max(pos + 2, NCH + 2 * (si - CUM1))
                add_event(agpos, lambda si=si: ag(g2_c, g2h_full, g2_full, OUT, si))
            # layer-2 post-slice work
            pos = NCH
            for si in range(NSL):
                pos = max(NCH + slice_avail1[si] + 3, pos + 1)
                add_event(min(pos, 2 * NCH - 1),
                          lambda si=si: l2_post_slice(si))

            # ---- run the stream -------------------------------------------
            for si in range(4):
                ag(g1_c, g1h_full, g1_full, HID, si)
            for ci, ch in enumerate(chunks):
                emit_chunk(1, g1_full, HID, acc1, ci, ch)
                for fn in events.get(ci, []):
                    fn()
            for ci, ch in enumerate(chunks):
                emit_chunk(2, g2_full, OUT, acc2, ci, ch)
                for fn in events.get(NCH + ci, []):
                    fn()
            for pos in sorted(k for k in events if k >= 2 * NCH):
                for fn in events[pos]:
                    fn()

    nc.compile()
    return nc


# ------------------------------------------------------------------ frontend

_CACHE = {}


def run(trace=False, **inputs):
    in_maps, meta = _host_prep(
        inputs["x"], inputs["edge_index"], inputs["W1"], inputs["b1"],
        inputs["W2"], inputs["b2"])
    key = (meta["N"], meta["IN_DIM"], meta["HID"], meta["OUT"], meta["TTOT"],
           tuple((s, p, n, tuple(sg)) for s, p, n, sg in meta["chunks"]))
    if key not in _CACHE:
        _CACHE.clear()
        _CACHE[key] = _build_program(meta)
    nc = _CACHE[key]
    res = run_bass_kernel_spmd(nc, in_maps, list(range(NC)), trace=trace)
    SH = meta["SH"]
    out = np.concatenate([res.results[c]["out"][:SH] for c in range(NC)],
                         axis=0)
    return out.astype(np.float32), res


def kernel(**inputs):
    out, _ = run(trace=False, **inputs)
    return out
